# revision 66
# baseline (speedup 1.0000x reference)
"""Trainium2 Bass kernel for nn_NodeEncoding_72816875537095.

Reference computation:
    scores = x @ W[0] + b[0]                          # [total]
    sp     = scatter(scores, pad_idx) -> [B, 96]      # padded per-graph scores
    num    = einsum('bijk,bk->bij', paths, sp)
    den    = paths.sum(-1) + 1e-8
    out    = num / den                                # [64, 96, 96]

Strategy (data-parallel over B across 8 NeuronCores, 8 graphs/core):
  - Host relayout: per core+graph, paths -> k-major [128, 9216] fp8 tiles
    (k rows 96..127 zero-padded: 0/1 exact in fp8, and 128-partition DMAs
    are ~2x faster than 96-partition ones, measured).
  - All 8 graph tiles are SBUF-resident (74KB/partition); every input DMA
    is issued up front on the SYNC HWDGE queue (the scalar queue starves
    behind sync bulk traffic, and a second bulk stream makes graph
    completions pairwise-late).  The ring is FIFO, so order = need:
    xt first (gates scores), then graphs 0..7, the last graph in halves
    so the tail pipeline stays fed.  W and bmask ride as extra columns
    of xt - tiny DMAs placed later in the ring would wait ~7us.
  - Scores on PE: per (graph, d-half), one [128,128] bf16 stationary tile
    of x^T (k on columns) x W-half [128,1] moving -> accumulated PSUM
    [128, 8] = scores in exactly the [k, g] layout the moving operand
    needs.  Rows 96..127 come out zero for free.
  - Main loop: per 128-column chunk of a graph, ONE matmul with the paths
    chunk as fp8 stationary (FWL-rate ~27ns/chunk steady-state) and a
    3-column moving operand [sp_hi, sp_lo, ones] -> PSUM [128, 3] =
    (num_hi, num_lo, den).  sp is hi/lo fp8-split (~8 mantissa bits;
    rel tolerance is 2e-2, measured end-to-end err ~1.9e-3).
  - Per-piece PSUM tile; epilogue is 5 wide strided ops: den+eps (ACT),
    rec (DVE reciprocal), hi (ACT copy out of PSUM), num = stt(lo/16+hi),
    out = num*rec.  Output is bf16 (host upcasts; error budget allows),
    stores staged (after g2, g5, end) ON THE SYNC RING: a scalar-queue
    store starves behind the sync stream and its recycled sem lane then
    blocks later paths triggers (measured 8us stall).
  - Output is stored partition-major [128, 576]; host un-permutes.

Wall-time budget (measured): ~7us fixed NEFF preamble, ~26us paths
stream at the 8-core-saturated HBM rate (~400GB/s/core), ~5us tail
(completion receipts + last-piece compute + store), ~8.5us fixed
framework teardown (semaphore-range clear).
"""

import sys

if "/opt/trn_rl_repo" not in sys.path:
    sys.path.insert(0, "/opt/trn_rl_repo")

import ml_dtypes
import numpy as np

import concourse.bass as bass  # noqa: F401
import concourse.mybir as mybir
from concourse import bacc, bass_utils
from concourse.tile import TileContext

F32 = mybir.dt.float32
BF16 = mybir.dt.bfloat16
FP8 = mybir.dt.float8e4
AF = mybir.ActivationFunctionType

B = 64
MAX_A = 96
D = 256
N_CORES = 8
G = B // N_CORES            # 8 graphs per core
COLS = MAX_A * MAX_A        # 9216
KP = 128                    # padded contraction rows
CHUNK = 128                 # stationary columns per matmul
CPG = COLS // CHUNK         # 72 chunks per graph
TOT = G * CPG               # 576 chunks per core
EPS = 1e-8
# per-graph paths DMA split (columns): graphs late in the stream arrive in
# smaller pieces so compute can chase the tail.
SPLITS = [1, 1, 1, 1, 1, 1, 1, 1]
# graph 7 streams in progressively smaller pieces: the LAST piece's DMA
# completion receipt + matmuls + epilogue + store are pure tail latency,
# so the smaller the final piece, the shorter the tail.
PIECES7 = [(0, 24), (24, 48), (48, 60), (60, 66), (66, 72)]
XCOLS = 2 * G * MAX_A + 2 + G     # xT halves (96 k-cols/graph) | W | bmask

_NC_CACHE = {}


def _build():
    if "nc" in _NC_CACHE:
        return _NC_CACHE["nc"]

    nc = bacc.Bacc("TRN2", target_bir_lowering=False, debug=False,
                   num_devices=N_CORES)

    pathsT_d = nc.dram_tensor("pathsT", [G, KP, COLS], FP8,
                              kind="ExternalInput")
    xt_d = nc.dram_tensor("xt", [KP, XCOLS], BF16, kind="ExternalInput")
    out_d = nc.dram_tensor("out", [CHUNK, TOT], BF16, kind="ExternalOutput")

    with TileContext(nc) as tc:
        with (
            tc.tile_pool(name="misc", bufs=1) as misc,
            tc.tile_pool(name="paths", bufs=8) as ppool,
            tc.tile_pool(name="spsum", bufs=1, space="PSUM") as sps,
            tc.tile_pool(name="psum", bufs=4, space="PSUM") as pspool,
            tc.tile_pool(name="epi", bufs=3) as epool,
        ):
            # ---- every input DMA up front, ALL on the sync queue: the
            # scalar queue's transfers starve behind sync-queue bulk
            # traffic (measured), and pairwise-interleaved paths streams
            # delay the first graphs.  xt (with W and bmask folded in as
            # extra columns) leads: it gates the scores pipeline. ----
            xt = misc.tile([KP, XCOLS], BF16)
            nc.sync.dma_start(out=xt[:], in_=xt_d[:])
            xw = 2 * G * MAX_A

            st = {}
            for g in range(G):
                st[g] = ppool.tile([KP, COLS], FP8, tag="st",
                                   name=f"st{g}")
                if g == G - 1:
                    for c0, c1 in PIECES7:
                        nc.sync.dma_start(
                            out=st[g][:, CHUNK * c0:CHUNK * c1],
                            in_=pathsT_d[g][:, CHUNK * c0:CHUNK * c1])
                    continue
                n = SPLITS[g]
                w = COLS // n
                for s in range(n):
                    nc.sync.dma_start(out=st[g][:, s * w:(s + 1) * w],
                                      in_=pathsT_d[g][:, s * w:(s + 1) * w])

            # ---- node scores on PE -> PSUM [96, G] (k-major) ----
            # 96-col stationary tiles write PSUM partitions 0..95 only;
            # w_all is fully zero-memset first so the unwritten PSUM rows
            # can never leak garbage into the fp8 weights.
            sp_ps = sps.tile([KP, G], F32, tag="sc")
            for g in range(G):
                for h in range(2):
                    nc.tensor.matmul(
                        sp_ps[0:MAX_A, g:g + 1],
                        lhsT=xt[:, (h * G + g) * MAX_A:
                                (h * G + g + 1) * MAX_A],
                        rhs=xt[:, xw + h:xw + h + 1],
                        start=(h == 0), stop=(h == 1))

            # w_all columns per graph g: [3g..3g+3) = [sp_hi, sp_lo*16, one]
            w_sp = misc.tile([MAX_A, G], F32)
            nc.vector.tensor_tensor(out=w_sp[:], in0=sp_ps[0:MAX_A, :],
                                    in1=xt[0:MAX_A, xw + 2:xw + 2 + G],
                                    op=mybir.AluOpType.add)
            w_hi = misc.tile([MAX_A, G], FP8)
            nc.vector.tensor_copy(w_hi[:], w_sp[:])
            r1 = misc.tile([MAX_A, G], F32)
            nc.vector.tensor_tensor(out=r1[:], in0=w_sp[:], in1=w_hi[:],
                                    op=mybir.AluOpType.subtract)
            w_all = misc.tile([KP, 3 * G], FP8)
            nc.vector.memset(w_all[:], 0.0)
            nc.vector.memset(w_all[:, 2:3 * G:3], 1.0)
            nc.vector.tensor_copy(w_all[0:MAX_A, 0:3 * G:3], w_hi[:])
            nc.vector.tensor_scalar_mul(out=w_all[0:MAX_A, 1:3 * G:3],
                                        in0=r1[:], scalar1=16.0)

            out_sb = misc.tile([CHUNK, TOT], BF16)

            # ---- main loop: one matmul per 128-column chunk ----
            # graph 7 is processed as three third-tiles so the tail (last
            # DMA -> last MM -> epilogue -> store) is as short as possible.
            pieces = [(g, 0, CPG) for g in range(G - 1)]
            pieces += [(G - 1, c0, c1) for c0, c1 in PIECES7]
            for g, c0, c1 in pieces:
                w = c1 - c0
                ps = pspool.tile([CHUNK, 3 * w], F32, tag="ps")
                for cl in range(c0, c1):
                    r = cl - c0
                    nc.tensor.matmul(
                        ps[:, 3 * r:3 * r + 3],
                        lhsT=st[g][:, CHUNK * cl:CHUNK * (cl + 1)],
                        rhs=w_all[:, 3 * g:3 * g + 3],
                        start=True, stop=True)
                # epilogue: out = (hi + lo/16) * 1/(den + eps)
                den = epool.tile([CHUNK, CPG], F32, tag="den")
                nc.scalar.activation(out=den[:, :w], in_=ps[:, 2:3 * w:3],
                                     func=AF.Copy, bias=EPS)
                rec = epool.tile([CHUNK, CPG], F32, tag="rec")
                nc.vector.reciprocal(out=rec[:, :w], in_=den[:, :w])
                hi = epool.tile([CHUNK, CPG], F32, tag="hi")
                nc.scalar.activation(out=hi[:, :w], in_=ps[:, 0:3 * w:3],
                                     func=AF.Copy)
                numt = epool.tile([CHUNK, CPG], F32, tag="numt")
                nc.vector.scalar_tensor_tensor(
                    out=numt[:, :w], in0=ps[:, 1:3 * w:3], scalar=0.0625,
                    in1=hi[:, :w],
                    op0=mybir.AluOpType.mult, op1=mybir.AluOpType.add)
                nc.vector.tensor_tensor(
                    out=out_sb[:, CPG * g + c0:CPG * g + c1],
                    in0=numt[:, :w], in1=rec[:, :w],
                    op=mybir.AluOpType.mult)
                # mid stores ride the SYNC ring too: a scalar-queue store
                # starves behind the sync stream, and its recycled sem lane
                # then blocks later paths triggers (measured: 8us stall).
                if g == 2 and c1 == CPG:
                    nc.sync.dma_start(out=out_d[:, :3 * CPG],
                                      in_=out_sb[:, :3 * CPG])
                if g == 5 and c1 == CPG:
                    nc.sync.dma_start(out=out_d[:, 3 * CPG:6 * CPG],
                                      in_=out_sb[:, 3 * CPG:6 * CPG])
                if g == 6 and c1 == CPG:
                    nc.sync.dma_start(out=out_d[:, 6 * CPG:7 * CPG],
                                      in_=out_sb[:, 6 * CPG:7 * CPG])
            nc.sync.dma_start(out=out_d[:, 7 * CPG:],
                              in_=out_sb[:, 7 * CPG:])

    nc.compile()
    _NC_CACHE["nc"] = nc
    return nc


def _host_prep(x, W, b, paths, pad_idx):
    x = np.ascontiguousarray(np.asarray(x, dtype=np.float32))
    W = np.asarray(W, dtype=np.float32)
    b = np.asarray(b, dtype=np.float32)
    pad_idx = np.asarray(pad_idx)

    # scatter x into padded [B*MAX_A, D] layout, mark valid slots
    xsc = np.zeros((B * MAX_A, D), dtype=np.float32)
    xsc[pad_idx] = x
    valid = np.zeros((B * MAX_A,), dtype=np.float32)
    valid[pad_idx] = 1.0
    bmask_full = (b[0] * valid).reshape(B, MAX_A)

    paths_f8 = np.asarray(paths).astype(ml_dtypes.float8_e4m3)

    in_maps = []
    for core in range(N_CORES):
        g0 = core * G
        pc = paths_f8[g0:g0 + G]  # [G, 96, 96, 96]
        pathsT = np.zeros((G, KP, COLS), dtype=ml_dtypes.float8_e4m3)
        pathsT[:, :MAX_A, :] = pc.transpose(0, 3, 1, 2).reshape(
            G, MAX_A, COLS)
        # xt[d, h*768 + g*96 + k] = x[g0+g, k, h*128 + d]
        # trailing columns: W halves (2 cols), bmask (G cols)
        xc = xsc[g0 * MAX_A:(g0 + G) * MAX_A].reshape(G, MAX_A, D)
        xthw = xc.transpose(2, 0, 1)                 # [D, G, 96]
        xt = np.zeros((KP, XCOLS), dtype=ml_dtypes.bfloat16)
        xt[:, :G * MAX_A] = xthw[:KP].reshape(KP, G * MAX_A)
        xt[:, G * MAX_A:2 * G * MAX_A] = xthw[KP:].reshape(KP, G * MAX_A)
        xw = 2 * G * MAX_A
        xt[:, xw] = W[0, :KP]
        xt[:, xw + 1] = W[0, KP:]
        xt[:MAX_A, xw + 2:] = bmask_full[g0:g0 + G].T
        in_maps.append({
            "pathsT": pathsT,
            "xt": xt,
        })
    return in_maps


LAST_RESULTS = None


def kernel(x, W, b, paths, pad_idx, _trace=False):
    global LAST_RESULTS
    nc = _build()
    in_maps = _host_prep(x, W, b, paths, pad_idx)
    res = bass_utils.run_bass_kernel_spmd(
        nc, in_maps, core_ids=list(range(N_CORES)), trace=_trace)
    LAST_RESULTS = res

    out = np.empty((B, MAX_A, MAX_A), dtype=np.float32)
    for core in range(N_CORES):
        oc = res.results[core]["out"]  # [128, 576] partition-major bf16
        out[core * G:(core + 1) * G] = (
            oc.T.reshape(G, MAX_A, MAX_A).astype(np.float32))
    return out


# revision 67
# speedup vs baseline: 1.0286x; 1.0286x over previous
"""Trainium2 Bass kernel for nn_NodeEncoding_72816875537095.

Reference computation:
    scores = x @ W[0] + b[0]                          # [total]
    sp     = scatter(scores, pad_idx) -> [B, 96]      # padded per-graph scores
    num    = einsum('bijk,bk->bij', paths, sp)
    den    = paths.sum(-1) + 1e-8
    out    = num / den                                # [64, 96, 96]

Strategy (data-parallel over B across 8 NeuronCores, 8 graphs/core):
  - Host relayout: per core+graph, paths -> k-major [128, 9216] fp8 tiles
    (k rows 96..127 zero-padded: 0/1 exact in fp8, and 128-partition DMAs
    are ~2x faster than 96-partition ones, measured).
  - All 8 graph tiles are SBUF-resident (74KB/partition); every input DMA
    is issued up front on the SYNC HWDGE queue (the scalar queue starves
    behind sync bulk traffic, and a second bulk stream makes graph
    completions pairwise-late).  The ring is FIFO, so order = need:
    xt first (gates scores), then graphs 0..7, the last graph in halves
    so the tail pipeline stays fed.  W and bmask ride as extra columns
    of xt - tiny DMAs placed later in the ring would wait ~7us.
  - Scores on PE: per (graph, d-half), one [128,128] bf16 stationary tile
    of x^T (k on columns) x W-half [128,1] moving -> accumulated PSUM
    [128, 8] = scores in exactly the [k, g] layout the moving operand
    needs.  Rows 96..127 come out zero for free.
  - Main loop: per 128-column chunk of a graph, ONE matmul with the paths
    chunk as fp8 stationary (FWL-rate ~27ns/chunk steady-state) and a
    3-column moving operand [sp_hi, sp_lo, ones] -> PSUM [128, 3] =
    (num_hi, num_lo, den).  sp is hi/lo fp8-split (~8 mantissa bits;
    rel tolerance is 2e-2, measured end-to-end err ~1.9e-3).
  - Per-piece PSUM tile; epilogue is 5 wide strided ops: den+eps (ACT),
    rec (DVE reciprocal), hi (ACT copy out of PSUM), num = stt(lo/16+hi),
    out = num*rec.  Output is bf16 (host upcasts; error budget allows),
    stores staged (after g2, g5, end) ON THE SYNC RING: a scalar-queue
    store starves behind the sync stream and its recycled sem lane then
    blocks later paths triggers (measured 8us stall).
  - Output is stored partition-major [128, 576]; host un-permutes.

Wall-time budget (measured): ~7us fixed NEFF preamble, ~26us paths
stream at the 8-core-saturated HBM rate (~400GB/s/core), ~5us tail
(completion receipts + last-piece compute + store), ~8.5us fixed
framework teardown (semaphore-range clear).
"""

import sys

if "/opt/trn_rl_repo" not in sys.path:
    sys.path.insert(0, "/opt/trn_rl_repo")

import ml_dtypes
import numpy as np

import concourse.bass as bass  # noqa: F401
import concourse.mybir as mybir
from concourse import bacc, bass_utils
from concourse.tile import TileContext

F32 = mybir.dt.float32
BF16 = mybir.dt.bfloat16
FP8 = mybir.dt.float8e4
AF = mybir.ActivationFunctionType

B = 64
MAX_A = 96
D = 256
N_CORES = 8
G = B // N_CORES            # 8 graphs per core
COLS = MAX_A * MAX_A        # 9216
KP = 128                    # padded contraction rows
CHUNK = 128                 # stationary columns per matmul
CPG = COLS // CHUNK         # 72 chunks per graph
TOT = G * CPG               # 576 chunks per core
EPS = 1e-8
# per-graph paths DMA split (columns): graphs late in the stream arrive in
# smaller pieces so compute can chase the tail.
SPLITS = [1, 1, 1, 1, 1, 2, 2, 1]
# graph 7 streams in progressively smaller pieces: the LAST piece's DMA
# completion receipt + matmuls + epilogue + store are pure tail latency,
# so the smaller the final piece, the shorter the tail.
PIECES7 = [(0, 24), (24, 48), (48, 60), (60, 66), (66, 72)]
XCOLS = 2 * G * MAX_A + 2 + G     # xT halves (96 k-cols/graph) | W | bmask

_NC_CACHE = {}


def _build():
    if "nc" in _NC_CACHE:
        return _NC_CACHE["nc"]

    nc = bacc.Bacc("TRN2", target_bir_lowering=False, debug=False,
                   num_devices=N_CORES)

    pathsT_d = nc.dram_tensor("pathsT", [G, KP, COLS], FP8,
                              kind="ExternalInput")
    xt_d = nc.dram_tensor("xt", [KP, XCOLS], BF16, kind="ExternalInput")
    out_d = nc.dram_tensor("out", [CHUNK, TOT], BF16, kind="ExternalOutput")

    with TileContext(nc) as tc:
        with (
            tc.tile_pool(name="misc", bufs=1) as misc,
            tc.tile_pool(name="paths", bufs=8) as ppool,
            tc.tile_pool(name="spsum", bufs=1, space="PSUM") as sps,
            tc.tile_pool(name="psum", bufs=4, space="PSUM") as pspool,
            tc.tile_pool(name="epi", bufs=3) as epool,
        ):
            # ---- every input DMA up front, ALL on the sync queue: the
            # scalar queue's transfers starve behind sync-queue bulk
            # traffic (measured), and pairwise-interleaved paths streams
            # delay the first graphs.  xt (with W and bmask folded in as
            # extra columns) leads: it gates the scores pipeline. ----
            xt = misc.tile([KP, XCOLS], BF16)
            nc.sync.dma_start(out=xt[:], in_=xt_d[:])
            xw = 2 * G * MAX_A

            st = {}
            for g in range(G):
                st[g] = ppool.tile([KP, COLS], FP8, tag="st",
                                   name=f"st{g}")
                if g == G - 1:
                    for c0, c1 in PIECES7:
                        nc.sync.dma_start(
                            out=st[g][:, CHUNK * c0:CHUNK * c1],
                            in_=pathsT_d[g][:, CHUNK * c0:CHUNK * c1])
                    continue
                n = SPLITS[g]
                w = COLS // n
                for s in range(n):
                    nc.sync.dma_start(out=st[g][:, s * w:(s + 1) * w],
                                      in_=pathsT_d[g][:, s * w:(s + 1) * w])

            # ---- node scores on PE -> PSUM [96, G] (k-major) ----
            # 96-col stationary tiles write PSUM partitions 0..95 only;
            # w_all is fully zero-memset first so the unwritten PSUM rows
            # can never leak garbage into the fp8 weights.
            sp_ps = sps.tile([KP, G], F32, tag="sc")
            for g in range(G):
                for h in range(2):
                    nc.tensor.matmul(
                        sp_ps[0:MAX_A, g:g + 1],
                        lhsT=xt[:, (h * G + g) * MAX_A:
                                (h * G + g + 1) * MAX_A],
                        rhs=xt[:, xw + h:xw + h + 1],
                        start=(h == 0), stop=(h == 1))

            # w_all columns per graph g: [3g..3g+3) = [sp_hi, sp_lo*16, one]
            w_sp = misc.tile([MAX_A, G], F32)
            nc.vector.tensor_tensor(out=w_sp[:], in0=sp_ps[0:MAX_A, :],
                                    in1=xt[0:MAX_A, xw + 2:xw + 2 + G],
                                    op=mybir.AluOpType.add)
            w_hi = misc.tile([MAX_A, G], FP8)
            nc.vector.tensor_copy(w_hi[:], w_sp[:])
            r1 = misc.tile([MAX_A, G], F32)
            nc.vector.tensor_tensor(out=r1[:], in0=w_sp[:], in1=w_hi[:],
                                    op=mybir.AluOpType.subtract)
            w_all = misc.tile([KP, 3 * G], FP8)
            nc.vector.memset(w_all[:], 0.0)
            nc.vector.memset(w_all[:, 2:3 * G:3], 1.0)
            nc.vector.tensor_copy(w_all[0:MAX_A, 0:3 * G:3], w_hi[:])
            nc.vector.tensor_scalar_mul(out=w_all[0:MAX_A, 1:3 * G:3],
                                        in0=r1[:], scalar1=16.0)

            out_sb = misc.tile([CHUNK, TOT], BF16)

            # ---- main loop: one matmul per 128-column chunk ----
            # graph 7 is processed as three third-tiles so the tail (last
            # DMA -> last MM -> epilogue -> store) is as short as possible.
            pieces = [(g, 0, CPG) for g in range(G - 1)]
            pieces += [(G - 1, c0, c1) for c0, c1 in PIECES7]
            for g, c0, c1 in pieces:
                w = c1 - c0
                ps = pspool.tile([CHUNK, 3 * w], F32, tag="ps")
                for cl in range(c0, c1):
                    r = cl - c0
                    nc.tensor.matmul(
                        ps[:, 3 * r:3 * r + 3],
                        lhsT=st[g][:, CHUNK * cl:CHUNK * (cl + 1)],
                        rhs=w_all[:, 3 * g:3 * g + 3],
                        start=True, stop=True)
                # epilogue: out = (hi + lo/16) * 1/(den + eps)
                den = epool.tile([CHUNK, CPG], F32, tag="den")
                nc.scalar.activation(out=den[:, :w], in_=ps[:, 2:3 * w:3],
                                     func=AF.Copy, bias=EPS)
                rec = epool.tile([CHUNK, CPG], F32, tag="rec")
                nc.vector.reciprocal(out=rec[:, :w], in_=den[:, :w])
                hi = epool.tile([CHUNK, CPG], F32, tag="hi")
                nc.scalar.activation(out=hi[:, :w], in_=ps[:, 0:3 * w:3],
                                     func=AF.Copy)
                numt = epool.tile([CHUNK, CPG], F32, tag="numt")
                nc.vector.scalar_tensor_tensor(
                    out=numt[:, :w], in0=ps[:, 1:3 * w:3], scalar=0.0625,
                    in1=hi[:, :w],
                    op0=mybir.AluOpType.mult, op1=mybir.AluOpType.add)
                nc.vector.tensor_tensor(
                    out=out_sb[:, CPG * g + c0:CPG * g + c1],
                    in0=numt[:, :w], in1=rec[:, :w],
                    op=mybir.AluOpType.mult)
                # mid stores ride the SYNC ring too: a scalar-queue store
                # starves behind the sync stream, and its recycled sem lane
                # then blocks later paths triggers (measured: 8us stall).
                if g == 2 and c1 == CPG:
                    nc.sync.dma_start(out=out_d[:, :3 * CPG],
                                      in_=out_sb[:, :3 * CPG])
                if g == 5 and c1 == CPG:
                    nc.sync.dma_start(out=out_d[:, 3 * CPG:6 * CPG],
                                      in_=out_sb[:, 3 * CPG:6 * CPG])
                if g == 6 and c1 == CPG:
                    nc.sync.dma_start(out=out_d[:, 6 * CPG:7 * CPG],
                                      in_=out_sb[:, 6 * CPG:7 * CPG])
            nc.sync.dma_start(out=out_d[:, 7 * CPG:],
                              in_=out_sb[:, 7 * CPG:])

    nc.compile()
    _NC_CACHE["nc"] = nc
    return nc


def _host_prep(x, W, b, paths, pad_idx):
    x = np.ascontiguousarray(np.asarray(x, dtype=np.float32))
    W = np.asarray(W, dtype=np.float32)
    b = np.asarray(b, dtype=np.float32)
    pad_idx = np.asarray(pad_idx)

    # scatter x into padded [B*MAX_A, D] layout, mark valid slots
    xsc = np.zeros((B * MAX_A, D), dtype=np.float32)
    xsc[pad_idx] = x
    valid = np.zeros((B * MAX_A,), dtype=np.float32)
    valid[pad_idx] = 1.0
    bmask_full = (b[0] * valid).reshape(B, MAX_A)

    paths_f8 = np.asarray(paths).astype(ml_dtypes.float8_e4m3)

    in_maps = []
    for core in range(N_CORES):
        g0 = core * G
        pc = paths_f8[g0:g0 + G]  # [G, 96, 96, 96]
        pathsT = np.zeros((G, KP, COLS), dtype=ml_dtypes.float8_e4m3)
        pathsT[:, :MAX_A, :] = pc.transpose(0, 3, 1, 2).reshape(
            G, MAX_A, COLS)
        # xt[d, h*768 + g*96 + k] = x[g0+g, k, h*128 + d]
        # trailing columns: W halves (2 cols), bmask (G cols)
        xc = xsc[g0 * MAX_A:(g0 + G) * MAX_A].reshape(G, MAX_A, D)
        xthw = xc.transpose(2, 0, 1)                 # [D, G, 96]
        xt = np.zeros((KP, XCOLS), dtype=ml_dtypes.bfloat16)
        xt[:, :G * MAX_A] = xthw[:KP].reshape(KP, G * MAX_A)
        xt[:, G * MAX_A:2 * G * MAX_A] = xthw[KP:].reshape(KP, G * MAX_A)
        xw = 2 * G * MAX_A
        xt[:, xw] = W[0, :KP]
        xt[:, xw + 1] = W[0, KP:]
        xt[:MAX_A, xw + 2:] = bmask_full[g0:g0 + G].T
        in_maps.append({
            "pathsT": pathsT,
            "xt": xt,
        })
    return in_maps


LAST_RESULTS = None


def kernel(x, W, b, paths, pad_idx, _trace=False):
    global LAST_RESULTS
    nc = _build()
    in_maps = _host_prep(x, W, b, paths, pad_idx)
    res = bass_utils.run_bass_kernel_spmd(
        nc, in_maps, core_ids=list(range(N_CORES)), trace=_trace)
    LAST_RESULTS = res

    out = np.empty((B, MAX_A, MAX_A), dtype=np.float32)
    for core in range(N_CORES):
        oc = res.results[core]["out"]  # [128, 576] partition-major bf16
        out[core * G:(core + 1) * G] = (
            oc.T.reshape(G, MAX_A, MAX_A).astype(np.float32))
    return out


# revision 69
# speedup vs baseline: 1.0464x; 1.0173x over previous
"""Trainium2 Bass kernel for nn_NodeEncoding_72816875537095.

Reference computation:
    scores = x @ W[0] + b[0]                          # [total]
    sp     = scatter(scores, pad_idx) -> [B, 96]      # padded per-graph scores
    num    = einsum('bijk,bk->bij', paths, sp)
    den    = paths.sum(-1) + 1e-8
    out    = num / den                                # [64, 96, 96]

Strategy (data-parallel over B across 8 NeuronCores, 8 graphs/core):
  - Host relayout: per core+graph, paths -> k-major [128, 9216] fp8 tiles
    (k rows 96..127 zero-padded: 0/1 exact in fp8, and 128-partition DMAs
    are ~2x faster than 96-partition ones, measured).
  - All 8 graph tiles are SBUF-resident (74KB/partition); every input DMA
    is issued up front on the SYNC HWDGE queue (the scalar queue starves
    behind sync bulk traffic, and a second bulk stream makes graph
    completions pairwise-late).  The ring is FIFO, so order = need:
    xt first (gates scores), then graphs 0..7, the last graph in halves
    so the tail pipeline stays fed.  W and bmask ride as extra columns
    of xt - tiny DMAs placed later in the ring would wait ~7us.
  - Scores on PE: per (graph, d-half), one [128,128] bf16 stationary tile
    of x^T (k on columns) x W-half [128,1] moving -> accumulated PSUM
    [128, 8] = scores in exactly the [k, g] layout the moving operand
    needs.  Rows 96..127 come out zero for free.
  - Main loop: per 128-column chunk of a graph, ONE matmul with the paths
    chunk as fp8 stationary (FWL-rate ~27ns/chunk steady-state) and a
    3-column moving operand [sp_hi, sp_lo, ones] -> PSUM [128, 3] =
    (num_hi, num_lo, den).  sp is hi/lo fp8-split (~8 mantissa bits;
    rel tolerance is 2e-2, measured end-to-end err ~1.9e-3).
  - Per-piece PSUM tile; epilogue is 5 wide strided ops: den+eps (ACT),
    rec (DVE reciprocal), hi (ACT copy out of PSUM), num = stt(lo/16+hi),
    out = num*rec.  Output is bf16 (host upcasts; error budget allows),
    stores staged (after g2, g5, end) ON THE SYNC RING: a scalar-queue
    store starves behind the sync stream and its recycled sem lane then
    blocks later paths triggers (measured 8us stall).
  - Output is stored partition-major [128, 576]; host un-permutes.

Wall-time budget (measured): ~7us fixed NEFF preamble, ~26us paths
stream at the 8-core-saturated HBM rate (~400GB/s/core), ~5us tail
(completion receipts + last-piece compute + store), ~8.5us fixed
framework teardown (semaphore-range clear).
"""

import sys

if "/opt/trn_rl_repo" not in sys.path:
    sys.path.insert(0, "/opt/trn_rl_repo")

import ml_dtypes
import numpy as np

import concourse.bass as bass  # noqa: F401
import concourse.mybir as mybir
from concourse import bacc, bass_utils
from concourse.tile import TileContext

F32 = mybir.dt.float32
BF16 = mybir.dt.bfloat16
FP8 = mybir.dt.float8e4
AF = mybir.ActivationFunctionType

B = 64
MAX_A = 96
D = 256
N_CORES = 8
G = B // N_CORES            # 8 graphs per core
COLS = MAX_A * MAX_A        # 9216
KP = 128                    # padded contraction rows
CHUNK = 128                 # stationary columns per matmul
CPG = COLS // CHUNK         # 72 chunks per graph
TOT = G * CPG               # 576 chunks per core
EPS = 1e-8
# per-graph paths DMA split (columns): graphs late in the stream arrive in
# smaller pieces so compute can chase the tail.
SPLITS = [1, 1, 1, 1, 1, 2, 2, 1]
# graph 7 streams in progressively smaller pieces: the LAST piece's DMA
# completion receipt + matmuls + epilogue + store are pure tail latency,
# so the smaller the final piece, the shorter the tail.
PIECES7 = [(0, 24), (24, 48), (48, 60), (60, 66), (66, 72)]
XCOLS = 2 * G * MAX_A + 2 + G     # xT halves (96 k-cols/graph) | W | bmask

_NC_CACHE = {}


def _build():
    if "nc" in _NC_CACHE:
        return _NC_CACHE["nc"]

    nc = bacc.Bacc("TRN2", target_bir_lowering=False, debug=False,
                   num_devices=N_CORES)

    pathsT_d = nc.dram_tensor("pathsT", [G, KP, COLS], FP8,
                              kind="ExternalInput")
    xt_d = nc.dram_tensor("xt", [KP, XCOLS], BF16, kind="ExternalInput")
    out_d = nc.dram_tensor("out", [CHUNK, TOT], BF16, kind="ExternalOutput")

    with TileContext(nc) as tc:
        with (
            tc.tile_pool(name="misc", bufs=1) as misc,
            tc.tile_pool(name="paths", bufs=8) as ppool,
            tc.tile_pool(name="spsum", bufs=1, space="PSUM") as sps,
            tc.tile_pool(name="psum", bufs=4, space="PSUM") as pspool,
            tc.tile_pool(name="epi", bufs=3) as epool,
        ):
            # ---- every input DMA up front, ALL on the sync queue: the
            # scalar queue's transfers starve behind sync-queue bulk
            # traffic (measured), and pairwise-interleaved paths streams
            # delay the first graphs.  xt (with W and bmask folded in as
            # extra columns) leads: it gates the scores pipeline. ----
            xt = misc.tile([KP, XCOLS], BF16)
            nc.sync.dma_start(out=xt[:], in_=xt_d[:])
            xw = 2 * G * MAX_A

            st = {}
            for g in range(G):
                st[g] = ppool.tile([KP, COLS], FP8, tag="st",
                                   name=f"st{g}")
                if g == G - 1:
                    for c0, c1 in PIECES7:
                        nc.sync.dma_start(
                            out=st[g][:, CHUNK * c0:CHUNK * c1],
                            in_=pathsT_d[g][:, CHUNK * c0:CHUNK * c1])
                    continue
                n = SPLITS[g]
                w = COLS // n
                for s in range(n):
                    nc.sync.dma_start(out=st[g][:, s * w:(s + 1) * w],
                                      in_=pathsT_d[g][:, s * w:(s + 1) * w])

            # ---- node scores on PE -> PSUM [96, G] (k-major) ----
            # 96-col stationary tiles write PSUM partitions 0..95 only;
            # w_all is fully zero-memset first so the unwritten PSUM rows
            # can never leak garbage into the fp8 weights.
            sp_ps = sps.tile([KP, G], F32, tag="sc")
            for g in range(G):
                for h in range(2):
                    nc.tensor.matmul(
                        sp_ps[0:MAX_A, g:g + 1],
                        lhsT=xt[:, (h * G + g) * MAX_A:
                                (h * G + g + 1) * MAX_A],
                        rhs=xt[:, xw + h:xw + h + 1],
                        start=(h == 0), stop=(h == 1))

            # w_all columns per graph g: [2g, 2g+1) = [sp (bf16), one].
            # bf16 moving operand against the fp8 stationary paths gives
            # ~8 mantissa bits of sp directly - no hi/lo split needed.
            w_sp = misc.tile([MAX_A, G], F32)
            nc.vector.tensor_tensor(out=w_sp[:], in0=sp_ps[0:MAX_A, :],
                                    in1=xt[0:MAX_A, xw + 2:xw + 2 + G],
                                    op=mybir.AluOpType.add)
            w_all = misc.tile([KP, 2 * G], BF16)
            nc.vector.memset(w_all[:], 0.0)
            nc.vector.memset(w_all[:, 1:2 * G:2], 1.0)
            nc.vector.tensor_copy(w_all[0:MAX_A, 0:2 * G:2], w_sp[:])

            out_sb = misc.tile([CHUNK, TOT], BF16)

            # ---- main loop: one matmul per 128-column chunk ----
            # graph 7 is processed as three third-tiles so the tail (last
            # DMA -> last MM -> epilogue -> store) is as short as possible.
            pieces = [(g, 0, CPG) for g in range(G - 1)]
            pieces += [(G - 1, c0, c1) for c0, c1 in PIECES7]
            for g, c0, c1 in pieces:
                w = c1 - c0
                ps = pspool.tile([CHUNK, 2 * w], F32, tag="ps")
                for cl in range(c0, c1):
                    r = cl - c0
                    nc.tensor.matmul(
                        ps[:, 2 * r:2 * r + 2],
                        lhsT=st[g][:, CHUNK * cl:CHUNK * (cl + 1)],
                        rhs=w_all[:, 2 * g:2 * g + 2],
                        start=True, stop=True)
                # epilogue: out = num * 1/(den + eps)
                den = epool.tile([CHUNK, CPG], F32, tag="den")
                nc.scalar.activation(out=den[:, :w], in_=ps[:, 1:2 * w:2],
                                     func=AF.Copy, bias=EPS)
                rec = epool.tile([CHUNK, CPG], F32, tag="rec")
                nc.vector.reciprocal(out=rec[:, :w], in_=den[:, :w])
                nc.vector.tensor_tensor(
                    out=out_sb[:, CPG * g + c0:CPG * g + c1],
                    in0=ps[:, 0:2 * w:2], in1=rec[:, :w],
                    op=mybir.AluOpType.mult)
                # mid stores ride the SYNC ring too: a scalar-queue store
                # starves behind the sync stream, and its recycled sem lane
                # then blocks later paths triggers (measured: 8us stall).
                if g == 2 and c1 == CPG:
                    nc.sync.dma_start(out=out_d[:, :3 * CPG],
                                      in_=out_sb[:, :3 * CPG])
                if g == 5 and c1 == CPG:
                    nc.sync.dma_start(out=out_d[:, 3 * CPG:6 * CPG],
                                      in_=out_sb[:, 3 * CPG:6 * CPG])
                if g == 6 and c1 == CPG:
                    nc.sync.dma_start(out=out_d[:, 6 * CPG:7 * CPG],
                                      in_=out_sb[:, 6 * CPG:7 * CPG])
            nc.sync.dma_start(out=out_d[:, 7 * CPG:],
                              in_=out_sb[:, 7 * CPG:])

    nc.compile()
    _NC_CACHE["nc"] = nc
    return nc


def _host_prep(x, W, b, paths, pad_idx):
    x = np.ascontiguousarray(np.asarray(x, dtype=np.float32))
    W = np.asarray(W, dtype=np.float32)
    b = np.asarray(b, dtype=np.float32)
    pad_idx = np.asarray(pad_idx)

    # scatter x into padded [B*MAX_A, D] layout, mark valid slots
    xsc = np.zeros((B * MAX_A, D), dtype=np.float32)
    xsc[pad_idx] = x
    valid = np.zeros((B * MAX_A,), dtype=np.float32)
    valid[pad_idx] = 1.0
    bmask_full = (b[0] * valid).reshape(B, MAX_A)

    paths_f8 = np.asarray(paths).astype(ml_dtypes.float8_e4m3)

    in_maps = []
    for core in range(N_CORES):
        g0 = core * G
        pc = paths_f8[g0:g0 + G]  # [G, 96, 96, 96]
        pathsT = np.zeros((G, KP, COLS), dtype=ml_dtypes.float8_e4m3)
        pathsT[:, :MAX_A, :] = pc.transpose(0, 3, 1, 2).reshape(
            G, MAX_A, COLS)
        # xt[d, h*768 + g*96 + k] = x[g0+g, k, h*128 + d]
        # trailing columns: W halves (2 cols), bmask (G cols)
        xc = xsc[g0 * MAX_A:(g0 + G) * MAX_A].reshape(G, MAX_A, D)
        xthw = xc.transpose(2, 0, 1)                 # [D, G, 96]
        xt = np.zeros((KP, XCOLS), dtype=ml_dtypes.bfloat16)
        xt[:, :G * MAX_A] = xthw[:KP].reshape(KP, G * MAX_A)
        xt[:, G * MAX_A:2 * G * MAX_A] = xthw[KP:].reshape(KP, G * MAX_A)
        xw = 2 * G * MAX_A
        xt[:, xw] = W[0, :KP]
        xt[:, xw + 1] = W[0, KP:]
        xt[:MAX_A, xw + 2:] = bmask_full[g0:g0 + G].T
        in_maps.append({
            "pathsT": pathsT,
            "xt": xt,
        })
    return in_maps


LAST_RESULTS = None


def kernel(x, W, b, paths, pad_idx, _trace=False):
    global LAST_RESULTS
    nc = _build()
    in_maps = _host_prep(x, W, b, paths, pad_idx)
    res = bass_utils.run_bass_kernel_spmd(
        nc, in_maps, core_ids=list(range(N_CORES)), trace=_trace)
    LAST_RESULTS = res

    out = np.empty((B, MAX_A, MAX_A), dtype=np.float32)
    for core in range(N_CORES):
        oc = res.results[core]["out"]  # [128, 576] partition-major bf16
        out[core * G:(core + 1) * G] = (
            oc.T.reshape(G, MAX_A, MAX_A).astype(np.float32))
    return out


# revision 70
# speedup vs baseline: 1.0533x; 1.0065x over previous
"""Trainium2 Bass kernel for nn_NodeEncoding_72816875537095.

Reference computation:
    scores = x @ W[0] + b[0]                          # [total]
    sp     = scatter(scores, pad_idx) -> [B, 96]      # padded per-graph scores
    num    = einsum('bijk,bk->bij', paths, sp)
    den    = paths.sum(-1) + 1e-8
    out    = num / den                                # [64, 96, 96]

Strategy (data-parallel over B across 8 NeuronCores, 8 graphs/core):
  - Host relayout: per core+graph, paths -> k-major [128, 9216] fp8 tiles
    (k rows 96..127 zero-padded: 0/1 exact in fp8, and 128-partition DMAs
    are ~2x faster than 96-partition ones, measured).
  - All 8 graph tiles are SBUF-resident (74KB/partition); every input DMA
    is issued up front on the SYNC HWDGE queue (the scalar queue starves
    behind sync bulk traffic, and a second bulk stream makes graph
    completions pairwise-late).  The ring is FIFO, so order = need:
    xt first (gates scores), then graphs 0..7, the last graph in halves
    so the tail pipeline stays fed.  W and bmask ride as extra columns
    of xt - tiny DMAs placed later in the ring would wait ~7us.
  - Scores on PE: per (graph, d-half), one [128,128] bf16 stationary tile
    of x^T (k on columns) x W-half [128,1] moving -> accumulated PSUM
    [128, 8] = scores in exactly the [k, g] layout the moving operand
    needs.  Rows 96..127 come out zero for free.
  - Main loop: per 128-column chunk of a graph, ONE matmul with the paths
    chunk as fp8 stationary (FWL-rate ~27ns/chunk steady-state) and a
    2-column BF16 moving operand [sp, ones] -> PSUM [128, 2] =
    (num, den).  Mixed fp8-stationary x bf16-moving is legal on TRN2 and
    gives ~8 mantissa bits of sp directly - no hi/lo fp8 split needed
    (rel tolerance is 2e-2, measured end-to-end err ~3.1e-3).
  - Per-piece PSUM tile; epilogue is 3 wide strided ops: den+eps (ACT),
    rec (DVE reciprocal), out = num*rec (DVE, PSUM x SBUF).  Output is
    bf16 (host upcasts; error budget allows), stores staged (after g2,
    g5, g6, end) ON THE SYNC RING: a scalar-queue store starves behind
    the sync stream and its recycled sem lane then blocks later paths
    triggers (measured 8us stall).
  - Output is stored partition-major [128, 576]; host un-permutes.

Wall-time budget (measured): ~7us fixed NEFF preamble, ~26us paths
stream at the 8-core-saturated HBM rate (~400GB/s/core), ~5us tail
(completion receipts + last-piece compute + store), ~8.5us fixed
framework teardown (semaphore-range clear).
"""

import sys

if "/opt/trn_rl_repo" not in sys.path:
    sys.path.insert(0, "/opt/trn_rl_repo")

import ml_dtypes
import numpy as np

import concourse.bass as bass  # noqa: F401
import concourse.mybir as mybir
from concourse import bacc, bass_utils
from concourse.tile import TileContext

F32 = mybir.dt.float32
BF16 = mybir.dt.bfloat16
FP8 = mybir.dt.float8e4
AF = mybir.ActivationFunctionType

B = 64
MAX_A = 96
D = 256
N_CORES = 8
G = B // N_CORES            # 8 graphs per core
COLS = MAX_A * MAX_A        # 9216
KP = 128                    # padded contraction rows
CHUNK = 128                 # stationary columns per matmul
CPG = COLS // CHUNK         # 72 chunks per graph
TOT = G * CPG               # 576 chunks per core
EPS = 1e-8
# per-graph paths DMA split (columns): graphs late in the stream arrive in
# smaller pieces so compute can chase the tail.
SPLITS = [1, 1, 1, 1, 1, 2, 2, 1]
# graph 7 streams in progressively smaller pieces: the LAST piece's DMA
# completion receipt + matmuls + epilogue + store are pure tail latency,
# so the smaller the final piece, the shorter the tail.
PIECES7 = [(0, 24), (24, 48), (48, 60), (60, 66), (66, 72)]
XCOLS = 2 * G * MAX_A + 2 + G     # xT halves (96 k-cols/graph) | W | bmask

_NC_CACHE = {}


def _build():
    if "nc" in _NC_CACHE:
        return _NC_CACHE["nc"]

    nc = bacc.Bacc("TRN2", target_bir_lowering=False, debug=False,
                   num_devices=N_CORES)

    pathsT_d = nc.dram_tensor("pathsT", [G, KP, COLS], FP8,
                              kind="ExternalInput")
    xt_d = nc.dram_tensor("xt", [KP, XCOLS], BF16, kind="ExternalInput")
    out_d = nc.dram_tensor("out", [CHUNK, TOT], BF16, kind="ExternalOutput")

    with TileContext(nc) as tc:
        with (
            tc.tile_pool(name="misc", bufs=1) as misc,
            tc.tile_pool(name="paths", bufs=8) as ppool,
            tc.tile_pool(name="spsum", bufs=1, space="PSUM") as sps,
            tc.tile_pool(name="psum", bufs=4, space="PSUM") as pspool,
            tc.tile_pool(name="epi", bufs=3) as epool,
        ):
            # ---- every input DMA up front, ALL on the sync queue: the
            # scalar queue's transfers starve behind sync-queue bulk
            # traffic (measured), and pairwise-interleaved paths streams
            # delay the first graphs.  xt (with W and bmask folded in as
            # extra columns) leads: it gates the scores pipeline. ----
            xt = misc.tile([KP, XCOLS], BF16)
            nc.sync.dma_start(out=xt[:], in_=xt_d[:])
            xw = 2 * G * MAX_A

            st = {}
            for g in range(G):
                st[g] = ppool.tile([KP, COLS], FP8, tag="st",
                                   name=f"st{g}")
                if g == G - 1:
                    for c0, c1 in PIECES7:
                        nc.sync.dma_start(
                            out=st[g][:, CHUNK * c0:CHUNK * c1],
                            in_=pathsT_d[g][:, CHUNK * c0:CHUNK * c1])
                    continue
                n = SPLITS[g]
                w = COLS // n
                for s in range(n):
                    nc.sync.dma_start(out=st[g][:, s * w:(s + 1) * w],
                                      in_=pathsT_d[g][:, s * w:(s + 1) * w])

            # ---- node scores on PE -> PSUM [96, G] (k-major) ----
            # 96-col stationary tiles write PSUM partitions 0..95 only;
            # w_all is fully zero-memset first so the unwritten PSUM rows
            # can never leak garbage into the fp8 weights.
            sp_ps = sps.tile([KP, G], F32, tag="sc")
            for g in range(G):
                for h in range(2):
                    nc.tensor.matmul(
                        sp_ps[0:MAX_A, g:g + 1],
                        lhsT=xt[:, (h * G + g) * MAX_A:
                                (h * G + g + 1) * MAX_A],
                        rhs=xt[:, xw + h:xw + h + 1],
                        start=(h == 0), stop=(h == 1))

            # w_all columns per graph g: [2g, 2g+1) = [sp (bf16), one].
            # bf16 moving operand against the fp8 stationary paths gives
            # ~8 mantissa bits of sp directly - no hi/lo split needed.
            w_sp = misc.tile([MAX_A, G], F32)
            nc.vector.tensor_tensor(out=w_sp[:], in0=sp_ps[0:MAX_A, :],
                                    in1=xt[0:MAX_A, xw + 2:xw + 2 + G],
                                    op=mybir.AluOpType.add)
            w_all = misc.tile([KP, 2 * G], BF16)
            nc.vector.memset(w_all[:], 0.0)
            nc.vector.memset(w_all[:, 1:2 * G:2], 1.0)
            nc.vector.tensor_copy(w_all[0:MAX_A, 0:2 * G:2], w_sp[:])

            out_sb = misc.tile([CHUNK, TOT], BF16)

            # ---- main loop: one matmul per 128-column chunk ----
            # graph 7 is processed as three third-tiles so the tail (last
            # DMA -> last MM -> epilogue -> store) is as short as possible.
            pieces = [(g, 0, CPG) for g in range(G - 1)]
            pieces += [(G - 1, c0, c1) for c0, c1 in PIECES7]
            for g, c0, c1 in pieces:
                w = c1 - c0
                ps = pspool.tile([CHUNK, 2 * w], F32, tag="ps")
                for cl in range(c0, c1):
                    r = cl - c0
                    nc.tensor.matmul(
                        ps[:, 2 * r:2 * r + 2],
                        lhsT=st[g][:, CHUNK * cl:CHUNK * (cl + 1)],
                        rhs=w_all[:, 2 * g:2 * g + 2],
                        start=True, stop=True)
                # epilogue: out = num * 1/(den + eps)
                den = epool.tile([CHUNK, CPG], F32, tag="den")
                nc.scalar.activation(out=den[:, :w], in_=ps[:, 1:2 * w:2],
                                     func=AF.Copy, bias=EPS)
                rec = epool.tile([CHUNK, CPG], F32, tag="rec")
                nc.vector.reciprocal(out=rec[:, :w], in_=den[:, :w])
                nc.vector.tensor_tensor(
                    out=out_sb[:, CPG * g + c0:CPG * g + c1],
                    in0=ps[:, 0:2 * w:2], in1=rec[:, :w],
                    op=mybir.AluOpType.mult)
                # mid stores ride the SYNC ring too: a scalar-queue store
                # starves behind the sync stream, and its recycled sem lane
                # then blocks later paths triggers (measured: 8us stall).
                if g == 2 and c1 == CPG:
                    nc.sync.dma_start(out=out_d[:, :3 * CPG],
                                      in_=out_sb[:, :3 * CPG])
                if g == 5 and c1 == CPG:
                    nc.sync.dma_start(out=out_d[:, 3 * CPG:6 * CPG],
                                      in_=out_sb[:, 3 * CPG:6 * CPG])
                if g == 6 and c1 == CPG:
                    nc.sync.dma_start(out=out_d[:, 6 * CPG:7 * CPG],
                                      in_=out_sb[:, 6 * CPG:7 * CPG])
            nc.sync.dma_start(out=out_d[:, 7 * CPG:],
                              in_=out_sb[:, 7 * CPG:])

    nc.compile()
    _NC_CACHE["nc"] = nc
    return nc


def _host_prep(x, W, b, paths, pad_idx):
    x = np.ascontiguousarray(np.asarray(x, dtype=np.float32))
    W = np.asarray(W, dtype=np.float32)
    b = np.asarray(b, dtype=np.float32)
    pad_idx = np.asarray(pad_idx)

    # scatter x into padded [B*MAX_A, D] layout, mark valid slots
    xsc = np.zeros((B * MAX_A, D), dtype=np.float32)
    xsc[pad_idx] = x
    valid = np.zeros((B * MAX_A,), dtype=np.float32)
    valid[pad_idx] = 1.0
    bmask_full = (b[0] * valid).reshape(B, MAX_A)

    paths_f8 = np.asarray(paths).astype(ml_dtypes.float8_e4m3)

    in_maps = []
    for core in range(N_CORES):
        g0 = core * G
        pc = paths_f8[g0:g0 + G]  # [G, 96, 96, 96]
        pathsT = np.zeros((G, KP, COLS), dtype=ml_dtypes.float8_e4m3)
        pathsT[:, :MAX_A, :] = pc.transpose(0, 3, 1, 2).reshape(
            G, MAX_A, COLS)
        # xt[d, h*768 + g*96 + k] = x[g0+g, k, h*128 + d]
        # trailing columns: W halves (2 cols), bmask (G cols)
        xc = xsc[g0 * MAX_A:(g0 + G) * MAX_A].reshape(G, MAX_A, D)
        xthw = xc.transpose(2, 0, 1)                 # [D, G, 96]
        xt = np.zeros((KP, XCOLS), dtype=ml_dtypes.bfloat16)
        xt[:, :G * MAX_A] = xthw[:KP].reshape(KP, G * MAX_A)
        xt[:, G * MAX_A:2 * G * MAX_A] = xthw[KP:].reshape(KP, G * MAX_A)
        xw = 2 * G * MAX_A
        xt[:, xw] = W[0, :KP]
        xt[:, xw + 1] = W[0, KP:]
        xt[:MAX_A, xw + 2:] = bmask_full[g0:g0 + G].T
        in_maps.append({
            "pathsT": pathsT,
            "xt": xt,
        })
    return in_maps


LAST_RESULTS = None


def kernel(x, W, b, paths, pad_idx, _trace=False):
    global LAST_RESULTS
    nc = _build()
    in_maps = _host_prep(x, W, b, paths, pad_idx)
    res = bass_utils.run_bass_kernel_spmd(
        nc, in_maps, core_ids=list(range(N_CORES)), trace=_trace)
    LAST_RESULTS = res

    out = np.empty((B, MAX_A, MAX_A), dtype=np.float32)
    for core in range(N_CORES):
        oc = res.results[core]["out"]  # [128, 576] partition-major bf16
        out[core * G:(core + 1) * G] = (
            oc.T.reshape(G, MAX_A, MAX_A).astype(np.float32))
    return out
